# revision 3
# baseline (speedup 1.0000x reference)
"""Trainium2 kernel for nn_AxialAttention_45749991637536.

Data-parallel across the flattened axial batch B = N*D*W = 896 -> 112 rows
per NeuronCore (spec sharding_hint), params replicated, exact BatchNorm
batch statistics via cross-device all-reduce (jax.lax.pmean).

v2: restructured for the Neuron backend to remove device transposes:
  - LayerNorm folded into the qkv / mlp1 matmuls (w' = w * ln_g on host;
    the per-position mean/rstd enter as rank-1 corrections), so no
    [B,C,H] <-> [B,H,C] swapaxes are materialized on device.
  - The stacked-BN + concat over [qk,qr,kr] is replaced by per-part BN
    statistics and one fused affine-sum (BN is per-channel affine, and
    transpose commutes with it), avoiding the [B,24,56,56] concat.
  - relative-embedding tables are expanded on host (input re-indexing).
"""

import numpy as np
import jax
import jax.numpy as jnp
from functools import partial

GROUPS = 8
EPS_LN = 1e-6
EPS_BN = 1e-5

# Hardcoded problem shapes (self-contained; do not read spec.json).
N, C, D, H, W = 2, 128, 8, 56, 56
NCORES = 8
B = N * D * W            # 896
BL = B // NCORES         # 112 per core
GP = C // GROUPS         # 16


def _shard_body(xb, w_qkv_g, s_qkv, t_qkv, bn_qkv_g, bn_qkv_b,
                bn_sim_g, bn_sim_b, q_emb, k_emb, v_emb, w_fc,
                w_mlp1_g, s_mlp, t_mlp, w_mlp2, use_psum):
    """Process one shard xb: [BL, C, H]. Runs under pmap axis 'cores'."""
    Bs = xb.shape[0]
    G = GROUPS
    gp = GP
    in_x = xb

    # ---- pre-norm over channels, folded into the qkv matmul ----
    # LN(x)[c] = (x-mu)*r*g[c] + b[c]; qkv = W @ LN(x)
    #          = r * (Wg @ x) - (r*mu) * s + t     (Wg, s, t host-precomputed)
    mu = jnp.mean(xb, axis=1, keepdims=True)                    # [B,1,H]
    var = jnp.mean(jnp.square(xb), axis=1, keepdims=True) - jnp.square(mu)
    r = jax.lax.rsqrt(var + EPS_LN)                             # [B,1,H]
    y = jnp.einsum('oc,bch->boh', w_qkv_g, xb)                  # [B,2C,H]
    qkv = r * y - (r * mu) * s_qkv[None, :, None] + t_qkv[None, :, None]

    # ---- BatchNorm1d over (B,H) with (optionally global) batch stats ----
    bmu = jnp.mean(qkv, axis=(0, 2))
    bm2 = jnp.mean(jnp.square(qkv), axis=(0, 2))
    if use_psum:
        bmu = jax.lax.pmean(bmu, axis_name='cores')
        bm2 = jax.lax.pmean(bm2, axis_name='cores')
    bvar = bm2 - jnp.square(bmu)
    brs = jax.lax.rsqrt(bvar + EPS_BN) * bn_qkv_g
    qkv = qkv * brs[None, :, None] + (bn_qkv_b - bmu * brs)[None, :, None]

    qkv = qkv.reshape(Bs, G, 2 * gp, H)
    q = qkv[:, :, : gp // 2]              # [B,G,8,H]
    k = qkv[:, :, gp // 2: gp]            # [B,G,8,H]
    v = qkv[:, :, gp:]                    # [B,G,16,H]

    # ---- similarity logits ----
    qr = jnp.einsum('bgci,cij->bgij', q, q_emb)
    # k_emb is host-transposed: k_emb[c,i,j] = all_emb_k[c,j,i], so this
    # equals swapaxes(einsum('bgci,cij->bgij', k, all_emb_k), 2, 3).
    kr = jnp.einsum('bgcj,cij->bgij', k, k_emb)
    qk = jnp.einsum('bgci,bgcj->bgij', q, k)

    # ---- BN over the stacked [qk,qr,kr] (channel axis 3G), fused ----
    # BN is per-channel affine; stats per part, then one affine-sum.
    def _stats(tt):
        m = jnp.mean(tt, axis=(0, 2, 3))
        m2 = jnp.mean(jnp.square(tt), axis=(0, 2, 3))
        return m, m2
    m_qk, m2_qk = _stats(qk)
    m_qr, m2_qr = _stats(qr)
    m_kr, m2_kr = _stats(kr)
    if use_psum:
        st = jnp.stack([m_qk, m_qr, m_kr, m2_qk, m2_qr, m2_kr])
        st = jax.lax.pmean(st, axis_name='cores')
        m_qk, m_qr, m_kr, m2_qk, m2_qr, m2_kr = st
    g_s = bn_sim_g.reshape(3, G)
    b_s = bn_sim_b.reshape(3, G)
    a1 = jax.lax.rsqrt(m2_qk - jnp.square(m_qk) + EPS_BN) * g_s[0]
    a2 = jax.lax.rsqrt(m2_qr - jnp.square(m_qr) + EPS_BN) * g_s[1]
    a3 = jax.lax.rsqrt(m2_kr - jnp.square(m_kr) + EPS_BN) * g_s[2]
    cc = (b_s[0] - m_qk * a1) + (b_s[1] - m_qr * a2) + (b_s[2] - m_kr * a3)
    logits = (qk * a1[None, :, None, None] + qr * a2[None, :, None, None]
              + kr * a3[None, :, None, None] + cc[None, :, None, None])

    similarity = jax.nn.softmax(logits, axis=3)   # [B,G,H,H]

    # ---- attend ----
    sv = jnp.einsum('bgij,bgcj->bgci', similarity, v)
    sve = jnp.einsum('bgij,cij->bgci', similarity, v_emb)
    so = jnp.concatenate([sv, sve], axis=-1).reshape(Bs, 2 * C, H)

    # fc branch: Linear(2C -> C), then raw .view(B, C, H)
    so = jnp.einsum('bch,oc->bho', so, w_fc)
    so = so.reshape(Bs, C, H)  # memory reinterpret, matches torch .view
    so = in_x + so

    # ---- mlp branch with shared LayerNorm, LN folded into mlp1 ----
    in2 = so
    mu2 = jnp.mean(so, axis=1, keepdims=True)
    var2 = jnp.mean(jnp.square(so), axis=1, keepdims=True) - jnp.square(mu2)
    r2 = jax.lax.rsqrt(var2 + EPS_LN)
    y2 = jnp.einsum('oc,bch->boh', w_mlp1_g, so)
    y2 = r2 * y2 - (r2 * mu2) * s_mlp[None, :, None] + t_mlp[None, :, None]
    y2 = jax.nn.relu(y2)                                        # [B,2C,H]
    out = jnp.einsum('co,boh->bch', w_mlp2, y2) + in2
    return out


_COMPILED = {}
_PARAM_CACHE = {}


def _get_compiled(use_psum):
    key = bool(use_psum)
    if key not in _COMPILED:
        fn = jax.pmap(
            partial(_shard_body, use_psum=key),
            axis_name='cores',
            in_axes=(0,) * 16,
            devices=jax.devices()[:NCORES],
        )
        _COMPILED[key] = fn
    return _COMPILED[key]


def _replicated_params(params):
    """Place the (small, replicated) parameter arrays on all 8 devices once;
    reuse across calls so only x is transferred per invocation."""
    key = "params"
    cached = _PARAM_CACHE.get(key)
    if cached is not None and all(
            np.array_equal(c_host, p) for c_host, p in zip(cached[0], params)):
        return cached[1]
    devs = jax.devices()[:NCORES]
    placed = tuple(
        jax.device_put_replicated(jnp.asarray(p, jnp.float32), devs)
        for p in params)
    _PARAM_CACHE[key] = ([np.asarray(p, np.float32) for p in params], placed)
    return placed


def _prep_params(w_qkv, bn_qkv_g, bn_qkv_b, ln_g, ln_b, bn_sim_g, bn_sim_b,
                 relative, w_fc, w_mlp1, w_mlp2):
    """Host-side precomputation of folded / expanded parameters."""
    w_qkv = np.asarray(w_qkv, np.float32)
    ln_g = np.asarray(ln_g, np.float32)
    ln_b = np.asarray(ln_b, np.float32)
    w_mlp1 = np.asarray(w_mlp1, np.float32)
    relative = np.asarray(relative, np.float32)

    w_qkv_g = w_qkv * ln_g[None, :]
    s_qkv = w_qkv_g.sum(axis=1)
    t_qkv = w_qkv @ ln_b
    w_mlp1_g = w_mlp1 * ln_g[None, :]
    s_mlp = w_mlp1_g.sum(axis=1)
    t_mlp = w_mlp1 @ ln_b

    # relative position embedding: all_emb[c,i,j] = relative[c, i-j+H-1]
    ar = np.arange(H)
    ridx = ar[:, None] - ar[None, :] + H - 1
    all_emb = relative[:, ridx]                  # [2gp, H, H]
    q_emb = all_emb[: GP // 2]
    k_emb = np.ascontiguousarray(all_emb[GP // 2: GP].transpose(0, 2, 1))
    v_emb = all_emb[GP:]

    return (w_qkv_g, s_qkv, t_qkv, bn_qkv_g, bn_qkv_b, bn_sim_g, bn_sim_b,
            q_emb, k_emb, v_emb, w_fc, w_mlp1_g, s_mlp, t_mlp, w_mlp2)


def kernel(x, w_qkv, bn_qkv_g, bn_qkv_b, ln_g, ln_b, bn_sim_g, bn_sim_b,
           relative, w_fc, w_mlp1, w_mlp2):
    x = np.asarray(x, dtype=np.float32)
    # [N,C,D,H,W] -> [N,D,W,C,H] -> [B, C, H], shard B over 8 cores
    xb = np.ascontiguousarray(
        np.transpose(x, (0, 2, 4, 1, 3))).reshape(B, C, H)
    xb_sh = xb.reshape(NCORES, BL, C, H)

    params = _replicated_params(_prep_params(
        w_qkv, bn_qkv_g, bn_qkv_b, ln_g, ln_b, bn_sim_g, bn_sim_b,
        relative, w_fc, w_mlp1, w_mlp2))
    args = (jnp.asarray(xb_sh),) + params

    try:
        out_sh = _get_compiled(True)(*args)
        out_sh = np.asarray(jax.device_get(out_sh))
    except Exception:
        # collectives unavailable -> per-shard BN stats (see sharding_hint)
        out_sh = np.asarray(jax.device_get(_get_compiled(False)(*args)))

    so = out_sh.reshape(B, C, H)
    out = so.reshape(N, D, W, C, H)
    return np.ascontiguousarray(np.transpose(out, (0, 3, 1, 4, 2)))


if __name__ == "__main__":
    import reference as R
    inp = R.setup_inputs()
    inp = {k: np.asarray(v) for k, v in inp.items()}
    out = kernel(**inp)
    print("kernel output:", out.shape, out.dtype)


# revision 5
# speedup vs baseline: 3.4679x; 3.4679x over previous
"""Trainium2 kernel for nn_AxialAttention_45749991637536.

Data-parallel across the flattened axial batch B = N*D*W = 896 -> 112 rows
per NeuronCore (spec sharding_hint), params replicated, exact BatchNorm
batch statistics via cross-device all-reduce (jax.lax.pmean).

v2: restructured for the Neuron backend to remove device transposes:
  - LayerNorm folded into the qkv / mlp1 matmuls (w' = w * ln_g on host;
    the per-position mean/rstd enter as rank-1 corrections), so no
    [B,C,H] <-> [B,H,C] swapaxes are materialized on device.
  - The stacked-BN + concat over [qk,qr,kr] is replaced by per-part BN
    statistics and one fused affine-sum (BN is per-channel affine, and
    transpose commutes with it), avoiding the [B,24,56,56] concat.
  - relative-embedding tables are expanded on host (input re-indexing).
"""

import numpy as np
import jax
import jax.numpy as jnp
from functools import partial

GROUPS = 8
EPS_LN = 1e-6
EPS_BN = 1e-5

# Hardcoded problem shapes (self-contained; do not read spec.json).
N, C, D, H, W = 2, 128, 8, 56, 56
NCORES = 8
B = N * D * W            # 896
BL = B // NCORES         # 112 per core
GP = C // GROUPS         # 16


def _shard_body(xb, w_qkv_g, s_qkv, t_qkv, bn_qkv_g, bn_qkv_b,
                bn_sim_g, bn_sim_b, q_emb, k_emb, v_emb, w_fc, w_fc_sve,
                w_mlp1_g, s_mlp, t_mlp, w_mlp2, use_psum):
    """Process one shard xb: [BL, C, H]. Runs under pmap axis 'cores'."""
    Bs = xb.shape[0]
    G = GROUPS
    gp = GP
    in_x = xb

    # ---- pre-norm over channels, folded into the qkv matmul ----
    # LN(x)[c] = (x-mu)*r*g[c] + b[c]; qkv = W @ LN(x)
    #          = r * (Wg @ x) - (r*mu) * s + t     (Wg, s, t host-precomputed)
    mu = jnp.mean(xb, axis=1, keepdims=True)                    # [B,1,H]
    var = jnp.mean(jnp.square(xb), axis=1, keepdims=True) - jnp.square(mu)
    r = jax.lax.rsqrt(var + EPS_LN)                             # [B,1,H]
    y = jnp.einsum('oc,bch->boh', w_qkv_g, xb)                  # [B,2C,H]
    qkv = r * y - (r * mu) * s_qkv[None, :, None] + t_qkv[None, :, None]

    # ---- BatchNorm1d over (B,H) with (optionally global) batch stats ----
    bmu = jnp.mean(qkv, axis=(0, 2))
    bm2 = jnp.mean(jnp.square(qkv), axis=(0, 2))
    if use_psum:
        bmu = jax.lax.pmean(bmu, axis_name='cores')
        bm2 = jax.lax.pmean(bm2, axis_name='cores')
    bvar = bm2 - jnp.square(bmu)
    brs = jax.lax.rsqrt(bvar + EPS_BN) * bn_qkv_g
    qkv = qkv * brs[None, :, None] + (bn_qkv_b - bmu * brs)[None, :, None]

    qkv = qkv.reshape(Bs, G, 2 * gp, H)
    q = qkv[:, :, : gp // 2]              # [B,G,8,H]
    k = qkv[:, :, gp // 2: gp]            # [B,G,8,H]
    v = qkv[:, :, gp:]                    # [B,G,16,H]

    # ---- similarity logits ----
    qr = jnp.einsum('bgci,cij->bgij', q, q_emb)
    # k_emb is host-transposed: k_emb[c,i,j] = all_emb_k[c,j,i], so this
    # equals swapaxes(einsum('bgci,cij->bgij', k, all_emb_k), 2, 3).
    kr = jnp.einsum('bgcj,cij->bgij', k, k_emb)
    qk = jnp.einsum('bgci,bgcj->bgij', q, k)

    # ---- BN over the stacked [qk,qr,kr] (channel axis 3G), fused ----
    # BN is per-channel affine; stats per part, then one affine-sum.
    def _stats(tt):
        m = jnp.mean(tt, axis=(0, 2, 3))
        m2 = jnp.mean(jnp.square(tt), axis=(0, 2, 3))
        return m, m2
    m_qk, m2_qk = _stats(qk)
    m_qr, m2_qr = _stats(qr)
    m_kr, m2_kr = _stats(kr)
    if use_psum:
        st = jnp.stack([m_qk, m_qr, m_kr, m2_qk, m2_qr, m2_kr])
        st = jax.lax.pmean(st, axis_name='cores')
        m_qk, m_qr, m_kr, m2_qk, m2_qr, m2_kr = st
    g_s = bn_sim_g.reshape(3, G)
    b_s = bn_sim_b.reshape(3, G)
    a1 = jax.lax.rsqrt(m2_qk - jnp.square(m_qk) + EPS_BN) * g_s[0]
    a2 = jax.lax.rsqrt(m2_qr - jnp.square(m_qr) + EPS_BN) * g_s[1]
    a3 = jax.lax.rsqrt(m2_kr - jnp.square(m_kr) + EPS_BN) * g_s[2]
    cc = (b_s[0] - m_qk * a1) + (b_s[1] - m_qr * a2) + (b_s[2] - m_kr * a3)
    logits = (qk * a1[None, :, None, None] + qr * a2[None, :, None, None]
              + kr * a3[None, :, None, None] + cc[None, :, None, None])

    similarity = jax.nn.softmax(logits, axis=3)   # [B,G,H,H]

    # ---- attend ----
    sv = jnp.einsum('bgij,bgcj->bgci', similarity, v)
    sve = jnp.einsum('bgij,cij->bgci', similarity, v_emb)

    # fc branch: Linear(2C -> C) on concat([sv,sve],-1).reshape(B,2C,H),
    # with the concat folded away: channel 32g+2c+half of the view holds
    # sv/sve[g,c], so w_fc is host-split (w_fc_sv/w_fc_sve: [C,G,gp]).
    so = (jnp.einsum('bgci,ogc->bio', sv, w_fc)
          + jnp.einsum('bgci,ogc->bio', sve, w_fc_sve))
    so = so.reshape(Bs, C, H)  # memory reinterpret, matches torch .view
    so = in_x + so

    # ---- mlp branch with shared LayerNorm, LN folded into mlp1 ----
    in2 = so
    mu2 = jnp.mean(so, axis=1, keepdims=True)
    var2 = jnp.mean(jnp.square(so), axis=1, keepdims=True) - jnp.square(mu2)
    r2 = jax.lax.rsqrt(var2 + EPS_LN)
    y2 = jnp.einsum('oc,bch->boh', w_mlp1_g, so)
    y2 = r2 * y2 - (r2 * mu2) * s_mlp[None, :, None] + t_mlp[None, :, None]
    y2 = jax.nn.relu(y2)                                        # [B,2C,H]
    out = jnp.einsum('co,boh->bch', w_mlp2, y2) + in2
    return out


_COMPILED = {}
_PARAM_CACHE = {}


def _get_compiled(use_psum):
    key = bool(use_psum)
    if key not in _COMPILED:
        fn = jax.pmap(
            partial(_shard_body, use_psum=key),
            axis_name='cores',
            in_axes=(0,) * 17,
            devices=jax.devices()[:NCORES],
        )
        _COMPILED[key] = fn
    return _COMPILED[key]


def _replicated_params(params):
    """Place the (small, replicated) parameter arrays on all 8 devices once;
    reuse across calls so only x is transferred per invocation."""
    key = "params"
    cached = _PARAM_CACHE.get(key)
    if cached is not None and all(
            np.array_equal(c_host, p) for c_host, p in zip(cached[0], params)):
        return cached[1]
    devs = jax.devices()[:NCORES]
    placed = tuple(
        jax.device_put_replicated(jnp.asarray(p, jnp.float32), devs)
        for p in params)
    _PARAM_CACHE[key] = ([np.asarray(p, np.float32) for p in params], placed)
    return placed


def _prep_params(w_qkv, bn_qkv_g, bn_qkv_b, ln_g, ln_b, bn_sim_g, bn_sim_b,
                 relative, w_fc, w_mlp1, w_mlp2):
    """Host-side precomputation of folded / expanded parameters."""
    w_qkv = np.asarray(w_qkv, np.float32)
    ln_g = np.asarray(ln_g, np.float32)
    ln_b = np.asarray(ln_b, np.float32)
    w_mlp1 = np.asarray(w_mlp1, np.float32)
    relative = np.asarray(relative, np.float32)

    w_qkv_g = w_qkv * ln_g[None, :]
    s_qkv = w_qkv_g.sum(axis=1)
    t_qkv = w_qkv @ ln_b
    w_mlp1_g = w_mlp1 * ln_g[None, :]
    s_mlp = w_mlp1_g.sum(axis=1)
    t_mlp = w_mlp1 @ ln_b

    # relative position embedding: all_emb[c,i,j] = relative[c, i-j+H-1]
    ar = np.arange(H)
    ridx = ar[:, None] - ar[None, :] + H - 1
    all_emb = relative[:, ridx]                  # [2gp, H, H]
    q_emb = all_emb[: GP // 2]
    k_emb = np.ascontiguousarray(all_emb[GP // 2: GP].transpose(0, 2, 1))
    v_emb = all_emb[GP:]

    w_fc = np.asarray(w_fc, np.float32).reshape(C, GROUPS, GP, 2)
    w_fc_sv = np.ascontiguousarray(w_fc[:, :, :, 0])
    w_fc_sve = np.ascontiguousarray(w_fc[:, :, :, 1])
    return (w_qkv_g, s_qkv, t_qkv, bn_qkv_g, bn_qkv_b, bn_sim_g, bn_sim_b,
            q_emb, k_emb, v_emb, w_fc_sv, w_fc_sve, w_mlp1_g, s_mlp, t_mlp,
            w_mlp2)


def kernel(x, w_qkv, bn_qkv_g, bn_qkv_b, ln_g, ln_b, bn_sim_g, bn_sim_b,
           relative, w_fc, w_mlp1, w_mlp2):
    x = np.asarray(x, dtype=np.float32)
    # [N,C,D,H,W] -> [N,D,W,C,H] -> [B, C, H], shard B over 8 cores
    xb = np.ascontiguousarray(
        np.transpose(x, (0, 2, 4, 1, 3))).reshape(B, C, H)
    xb_sh = xb.reshape(NCORES, BL, C, H)

    params = _replicated_params(_prep_params(
        w_qkv, bn_qkv_g, bn_qkv_b, ln_g, ln_b, bn_sim_g, bn_sim_b,
        relative, w_fc, w_mlp1, w_mlp2))
    args = (jnp.asarray(xb_sh),) + params

    try:
        out_sh = _get_compiled(True)(*args)
        out_sh = np.asarray(jax.device_get(out_sh))
    except Exception:
        # collectives unavailable -> per-shard BN stats (see sharding_hint)
        out_sh = np.asarray(jax.device_get(_get_compiled(False)(*args)))

    so = out_sh.reshape(B, C, H)
    out = so.reshape(N, D, W, C, H)
    return np.ascontiguousarray(np.transpose(out, (0, 3, 1, 4, 2)))


if __name__ == "__main__":
    import reference as R
    inp = R.setup_inputs()
    inp = {k: np.asarray(v) for k, v in inp.items()}
    out = kernel(**inp)
    print("kernel output:", out.shape, out.dtype)


# revision 7
# speedup vs baseline: 4.5765x; 1.3197x over previous
"""Trainium2 kernel for nn_AxialAttention_45749991637536.

Data-parallel across the flattened axial batch B = N*D*W = 896 -> 112 rows
per NeuronCore (spec sharding_hint), params replicated, exact BatchNorm
batch statistics via cross-device all-reduce (jax.lax.pmean).

v2: restructured for the Neuron backend to remove device transposes:
  - LayerNorm folded into the qkv / mlp1 matmuls (w' = w * ln_g on host;
    the per-position mean/rstd enter as rank-1 corrections), so no
    [B,C,H] <-> [B,H,C] swapaxes are materialized on device.
  - The stacked-BN + concat over [qk,qr,kr] is replaced by per-part BN
    statistics and one fused affine-sum (BN is per-channel affine, and
    transpose commutes with it), avoiding the [B,24,56,56] concat.
  - relative-embedding tables are expanded on host (input re-indexing).
"""

import numpy as np
import jax
import jax.numpy as jnp
from functools import partial

GROUPS = 8
EPS_LN = 1e-6
EPS_BN = 1e-5

# Hardcoded problem shapes (self-contained; do not read spec.json).
N, C, D, H, W = 2, 128, 8, 56, 56
NCORES = 8
B = N * D * W            # 896
BL = B // NCORES         # 112 per core
GP = C // GROUPS         # 16


def _shard_body(xb, w_qkv_g, s_qkv, t_qkv, bn_qkv_g, bn_qkv_b,
                bn_sim_g, bn_sim_b, q_emb, k_emb, v_emb, w_fc, w_fc_sve,
                w_mlp1_g, s_mlp, t_mlp, w_mlp2, use_psum):
    """Process one shard xb: [BL, C, H]. Runs under pmap axis 'cores'."""
    Bs = xb.shape[0]
    G = GROUPS
    gp = GP
    in_x = xb

    # ---- pre-norm over channels, folded into the qkv matmul ----
    # LN(x)[c] = (x-mu)*r*g[c] + b[c]; qkv = W @ LN(x)
    #          = r * (Wg @ x) - (r*mu) * s + t     (Wg, s, t host-precomputed)
    mu = jnp.mean(xb, axis=1, keepdims=True)                    # [B,1,H]
    var = jnp.mean(jnp.square(xb), axis=1, keepdims=True) - jnp.square(mu)
    r = jax.lax.rsqrt(var + EPS_LN)                             # [B,1,H]
    y = jnp.einsum('oc,bch->boh', w_qkv_g, xb)                  # [B,2C,H]
    qkv = r * y - (r * mu) * s_qkv[None, :, None] + t_qkv[None, :, None]

    # ---- BatchNorm1d over (B,H) with (optionally global) batch stats ----
    bmu = jnp.mean(qkv, axis=(0, 2))
    bm2 = jnp.mean(jnp.square(qkv), axis=(0, 2))
    if use_psum:
        bmu = jax.lax.pmean(bmu, axis_name='cores')
        bm2 = jax.lax.pmean(bm2, axis_name='cores')
    bvar = bm2 - jnp.square(bmu)
    brs = jax.lax.rsqrt(bvar + EPS_BN) * bn_qkv_g
    qkv = qkv * brs[None, :, None] + (bn_qkv_b - bmu * brs)[None, :, None]

    qkv = qkv.reshape(Bs, G, 2 * gp, H)
    bf = jnp.bfloat16
    q = qkv[:, :, : gp // 2].astype(bf)   # [B,G,8,H]
    k = qkv[:, :, gp // 2: gp].astype(bf)
    v = qkv[:, :, gp:].astype(bf)         # [B,G,16,H]

    # ---- similarity logits (bf16 storage, fp32 accumulation) ----
    qr = jnp.einsum('bgci,cij->bgij', q, q_emb.astype(bf))
    # k_emb is host-transposed: k_emb[c,i,j] = all_emb_k[c,j,i], so this
    # equals swapaxes(einsum('bgci,cij->bgij', k, all_emb_k), 2, 3).
    kr = jnp.einsum('bgcj,cij->bgij', k, k_emb.astype(bf))
    qk = jnp.einsum('bgci,bgcj->bgij', q, k)

    # ---- BN over the stacked [qk,qr,kr] (channel axis 3G), fused ----
    # BN is per-channel affine; stats per part, then one affine-sum.
    def _stats(tt):
        m = jnp.mean(tt, axis=(0, 2, 3), dtype=jnp.float32)
        m2 = jnp.mean(jnp.square(tt.astype(jnp.float32)), axis=(0, 2, 3))
        return m, m2
    m_qk, m2_qk = _stats(qk)
    m_qr, m2_qr = _stats(qr)
    m_kr, m2_kr = _stats(kr)
    if use_psum:
        st = jnp.stack([m_qk, m_qr, m_kr, m2_qk, m2_qr, m2_kr])
        st = jax.lax.pmean(st, axis_name='cores')
        m_qk, m_qr, m_kr, m2_qk, m2_qr, m2_kr = st
    g_s = bn_sim_g.reshape(3, G)
    b_s = bn_sim_b.reshape(3, G)
    a1 = jax.lax.rsqrt(m2_qk - jnp.square(m_qk) + EPS_BN) * g_s[0]
    a2 = jax.lax.rsqrt(m2_qr - jnp.square(m_qr) + EPS_BN) * g_s[1]
    a3 = jax.lax.rsqrt(m2_kr - jnp.square(m_kr) + EPS_BN) * g_s[2]
    cc = (b_s[0] - m_qk * a1) + (b_s[1] - m_qr * a2) + (b_s[2] - m_kr * a3)
    logits = (qk * a1[None, :, None, None] + qr * a2[None, :, None, None]
              + kr * a3[None, :, None, None] + cc[None, :, None, None])

    # softmax without max-subtraction: logits are BN-standardized per
    # channel (unit variance), so exp cannot overflow in fp32.
    e = jnp.exp(logits)
    similarity = e / jnp.sum(e, axis=3, keepdims=True)   # [B,G,H,H] fp32

    # ---- attend ----
    sim_h = similarity.astype(bf)
    sv = jnp.einsum('bgij,bgcj->bgci', sim_h, v)
    sve = jnp.einsum('bgij,cij->bgci', sim_h, v_emb.astype(bf))

    # fc branch: Linear(2C -> C) on concat([sv,sve],-1).reshape(B,2C,H),
    # with the concat folded away: channel 32g+2c+half of the view holds
    # sv/sve[g,c], so w_fc is host-split (w_fc_sv/w_fc_sve: [C,G,gp]).
    so = (jnp.einsum('bgci,ogc->bio', sv, w_fc)
          + jnp.einsum('bgci,ogc->bio', sve, w_fc_sve))
    so = so.reshape(Bs, C, H)  # memory reinterpret, matches torch .view
    so = in_x + so

    # ---- mlp branch with shared LayerNorm, LN folded into mlp1 ----
    in2 = so
    mu2 = jnp.mean(so, axis=1, keepdims=True)
    var2 = jnp.mean(jnp.square(so), axis=1, keepdims=True) - jnp.square(mu2)
    r2 = jax.lax.rsqrt(var2 + EPS_LN)
    y2 = jnp.einsum('oc,bch->boh', w_mlp1_g, so)
    y2 = r2 * y2 - (r2 * mu2) * s_mlp[None, :, None] + t_mlp[None, :, None]
    y2 = jax.nn.relu(y2)                                        # [B,2C,H]
    out = jnp.einsum('co,boh->bch', w_mlp2, y2) + in2
    return out


_COMPILED = {}
_PARAM_CACHE = {}


def _get_compiled(use_psum):
    key = bool(use_psum)
    if key not in _COMPILED:
        fn = jax.pmap(
            partial(_shard_body, use_psum=key),
            axis_name='cores',
            in_axes=(0,) * 17,
            devices=jax.devices()[:NCORES],
        )
        _COMPILED[key] = fn
    return _COMPILED[key]


def _replicated_params(params):
    """Place the (small, replicated) parameter arrays on all 8 devices once;
    reuse across calls so only x is transferred per invocation."""
    key = "params"
    cached = _PARAM_CACHE.get(key)
    if cached is not None and all(
            np.array_equal(c_host, p) for c_host, p in zip(cached[0], params)):
        return cached[1]
    devs = jax.devices()[:NCORES]
    placed = tuple(
        jax.device_put_replicated(jnp.asarray(p, jnp.float32), devs)
        for p in params)
    _PARAM_CACHE[key] = ([np.asarray(p, np.float32) for p in params], placed)
    return placed


def _prep_params(w_qkv, bn_qkv_g, bn_qkv_b, ln_g, ln_b, bn_sim_g, bn_sim_b,
                 relative, w_fc, w_mlp1, w_mlp2):
    """Host-side precomputation of folded / expanded parameters."""
    w_qkv = np.asarray(w_qkv, np.float32)
    ln_g = np.asarray(ln_g, np.float32)
    ln_b = np.asarray(ln_b, np.float32)
    w_mlp1 = np.asarray(w_mlp1, np.float32)
    relative = np.asarray(relative, np.float32)

    w_qkv_g = w_qkv * ln_g[None, :]
    s_qkv = w_qkv_g.sum(axis=1)
    t_qkv = w_qkv @ ln_b
    w_mlp1_g = w_mlp1 * ln_g[None, :]
    s_mlp = w_mlp1_g.sum(axis=1)
    t_mlp = w_mlp1 @ ln_b

    # relative position embedding: all_emb[c,i,j] = relative[c, i-j+H-1]
    ar = np.arange(H)
    ridx = ar[:, None] - ar[None, :] + H - 1
    all_emb = relative[:, ridx]                  # [2gp, H, H]
    q_emb = all_emb[: GP // 2]
    k_emb = np.ascontiguousarray(all_emb[GP // 2: GP].transpose(0, 2, 1))
    v_emb = all_emb[GP:]

    w_fc = np.asarray(w_fc, np.float32).reshape(C, GROUPS, GP, 2)
    w_fc_sv = np.ascontiguousarray(w_fc[:, :, :, 0])
    w_fc_sve = np.ascontiguousarray(w_fc[:, :, :, 1])
    return (w_qkv_g, s_qkv, t_qkv, bn_qkv_g, bn_qkv_b, bn_sim_g, bn_sim_b,
            q_emb, k_emb, v_emb, w_fc_sv, w_fc_sve, w_mlp1_g, s_mlp, t_mlp,
            w_mlp2)


def kernel(x, w_qkv, bn_qkv_g, bn_qkv_b, ln_g, ln_b, bn_sim_g, bn_sim_b,
           relative, w_fc, w_mlp1, w_mlp2):
    x = np.asarray(x, dtype=np.float32)
    # [N,C,D,H,W] -> [N,D,W,C,H] -> [B, C, H], shard B over 8 cores
    xb = np.ascontiguousarray(
        np.transpose(x, (0, 2, 4, 1, 3))).reshape(B, C, H)
    xb_sh = xb.reshape(NCORES, BL, C, H)

    params = _replicated_params(_prep_params(
        w_qkv, bn_qkv_g, bn_qkv_b, ln_g, ln_b, bn_sim_g, bn_sim_b,
        relative, w_fc, w_mlp1, w_mlp2))
    args = (jnp.asarray(xb_sh),) + params

    try:
        out_sh = _get_compiled(True)(*args)
        out_sh = np.asarray(jax.device_get(out_sh))
    except Exception:
        # collectives unavailable -> per-shard BN stats (see sharding_hint)
        out_sh = np.asarray(jax.device_get(_get_compiled(False)(*args)))

    so = out_sh.reshape(B, C, H)
    out = so.reshape(N, D, W, C, H)
    return np.ascontiguousarray(np.transpose(out, (0, 3, 1, 4, 2)))


if __name__ == "__main__":
    import reference as R
    inp = R.setup_inputs()
    inp = {k: np.asarray(v) for k, v in inp.items()}
    out = kernel(**inp)
    print("kernel output:", out.shape, out.dtype)


# revision 8
# speedup vs baseline: 5.9485x; 1.2998x over previous
"""Trainium2 kernel for nn_AxialAttention_45749991637536.

Data-parallel across the flattened axial batch B = N*D*W = 896 -> 112 rows
per NeuronCore (spec sharding_hint), params replicated, exact BatchNorm
batch statistics via cross-device all-reduce (jax.lax.pmean).

v2: restructured for the Neuron backend to remove device transposes:
  - LayerNorm folded into the qkv / mlp1 matmuls (w' = w * ln_g on host;
    the per-position mean/rstd enter as rank-1 corrections), so no
    [B,C,H] <-> [B,H,C] swapaxes are materialized on device.
  - The stacked-BN + concat over [qk,qr,kr] is replaced by per-part BN
    statistics and one fused affine-sum (BN is per-channel affine, and
    transpose commutes with it), avoiding the [B,24,56,56] concat.
  - relative-embedding tables are expanded on host (input re-indexing).
"""

import numpy as np
import jax
import jax.numpy as jnp
from functools import partial

GROUPS = 8
EPS_LN = 1e-6
EPS_BN = 1e-5

# Hardcoded problem shapes (self-contained; do not read spec.json).
N, C, D, H, W = 2, 128, 8, 56, 56
NCORES = 8
B = N * D * W            # 896
BL = B // NCORES         # 112 per core
GP = C // GROUPS         # 16


def _shard_body(xb, w_qkv_g, s_qkv, t_qkv, bn_qkv_g, bn_qkv_b,
                bn_sim_g, bn_sim_b, q_emb, k_emb, v_emb, w_fc, w_fc_sve,
                w_mlp1_g, s_mlp, t_mlp, w_mlp2, use_psum):
    """Process one shard xb: [BL, C, H]. Runs under pmap axis 'cores'."""
    Bs = xb.shape[0]
    G = GROUPS
    gp = GP
    in_x = xb

    # ---- pre-norm over channels, folded into the qkv matmul ----
    # LN(x)[c] = (x-mu)*r*g[c] + b[c]; qkv = W @ LN(x)
    #          = r * (Wg @ x) - (r*mu) * s + t     (Wg, s, t host-precomputed)
    mu = jnp.mean(xb, axis=1, keepdims=True)                    # [B,1,H]
    var = jnp.mean(jnp.square(xb), axis=1, keepdims=True) - jnp.square(mu)
    r = jax.lax.rsqrt(var + EPS_LN)                             # [B,1,H]
    bf = jnp.bfloat16
    y = jnp.einsum('oc,bch->boh', w_qkv_g.astype(bf), xb.astype(bf),
                   preferred_element_type=jnp.float32)          # [B,2C,H]
    qkv = r * y - (r * mu) * s_qkv[None, :, None] + t_qkv[None, :, None]

    # ---- BatchNorm1d over (B,H) with (optionally global) batch stats ----
    bmu = jnp.mean(qkv, axis=(0, 2))
    bm2 = jnp.mean(jnp.square(qkv), axis=(0, 2))
    if use_psum:
        bmu = jax.lax.pmean(bmu, axis_name='cores')
        bm2 = jax.lax.pmean(bm2, axis_name='cores')
    bvar = bm2 - jnp.square(bmu)
    brs = jax.lax.rsqrt(bvar + EPS_BN) * bn_qkv_g
    qkv = qkv * brs[None, :, None] + (bn_qkv_b - bmu * brs)[None, :, None]

    qkv = qkv.reshape(Bs, G, 2 * gp, H)
    q = qkv[:, :, : gp // 2].astype(bf)   # [B,G,8,H]
    k = qkv[:, :, gp // 2: gp].astype(bf)
    v = qkv[:, :, gp:].astype(bf)         # [B,G,16,H]

    # ---- similarity logits (bf16 storage, fp32 accumulation) ----
    qr = jnp.einsum('bgci,cij->bgij', q, q_emb.astype(bf))
    # k_emb is host-transposed: k_emb[c,i,j] = all_emb_k[c,j,i], so this
    # equals swapaxes(einsum('bgci,cij->bgij', k, all_emb_k), 2, 3).
    kr = jnp.einsum('bgcj,cij->bgij', k, k_emb.astype(bf))
    qk = jnp.einsum('bgci,bgcj->bgij', q, k)

    # ---- BN over the stacked [qk,qr,kr] (channel axis 3G), fused ----
    # BN is per-channel affine; stats per part, then one affine-sum.
    def _stats(tt):
        m = jnp.mean(tt, axis=(0, 2, 3), dtype=jnp.float32)
        m2 = jnp.mean(jnp.square(tt.astype(jnp.float32)), axis=(0, 2, 3))
        return m, m2
    m_qk, m2_qk = _stats(qk)
    m_qr, m2_qr = _stats(qr)
    m_kr, m2_kr = _stats(kr)
    if use_psum:
        st = jnp.stack([m_qk, m_qr, m_kr, m2_qk, m2_qr, m2_kr])
        st = jax.lax.pmean(st, axis_name='cores')
        m_qk, m_qr, m_kr, m2_qk, m2_qr, m2_kr = st
    g_s = bn_sim_g.reshape(3, G)
    b_s = bn_sim_b.reshape(3, G)
    a1 = jax.lax.rsqrt(m2_qk - jnp.square(m_qk) + EPS_BN) * g_s[0]
    a2 = jax.lax.rsqrt(m2_qr - jnp.square(m_qr) + EPS_BN) * g_s[1]
    a3 = jax.lax.rsqrt(m2_kr - jnp.square(m_kr) + EPS_BN) * g_s[2]
    cc = (b_s[0] - m_qk * a1) + (b_s[1] - m_qr * a2) + (b_s[2] - m_kr * a3)
    logits = (qk * a1.astype(bf)[None, :, None, None]
              + qr * a2.astype(bf)[None, :, None, None]
              + kr * a3.astype(bf)[None, :, None, None]
              + cc.astype(bf)[None, :, None, None])

    # softmax without max-subtraction: logits are BN-standardized per
    # channel (unit variance), so exp cannot overflow in fp32.
    e = jnp.exp(logits.astype(jnp.float32))
    similarity = e / jnp.sum(e, axis=3, keepdims=True)   # [B,G,H,H] fp32

    # ---- attend ----
    sim_h = similarity.astype(bf)
    sv = jnp.einsum('bgij,bgcj->bgci', sim_h, v)
    sve = jnp.einsum('bgij,cij->bgci', sim_h, v_emb.astype(bf))

    # fc branch: Linear(2C -> C) on concat([sv,sve],-1).reshape(B,2C,H),
    # with the concat folded away: channel 32g+2c+half of the view holds
    # sv/sve[g,c], so w_fc is host-split (w_fc_sv/w_fc_sve: [C,G,gp]).
    so = (jnp.einsum('bgci,ogc->bio', sv, w_fc.astype(bf),
                     preferred_element_type=jnp.float32)
          + jnp.einsum('bgci,ogc->bio', sve, w_fc_sve.astype(bf),
                       preferred_element_type=jnp.float32))
    so = so.reshape(Bs, C, H)  # memory reinterpret, matches torch .view
    so = in_x + so

    # ---- mlp branch with shared LayerNorm, LN folded into mlp1 ----
    in2 = so
    mu2 = jnp.mean(so, axis=1, keepdims=True)
    var2 = jnp.mean(jnp.square(so), axis=1, keepdims=True) - jnp.square(mu2)
    r2 = jax.lax.rsqrt(var2 + EPS_LN)
    y2 = jnp.einsum('oc,bch->boh', w_mlp1_g.astype(bf), so.astype(bf),
                    preferred_element_type=jnp.float32)
    y2 = r2 * y2 - (r2 * mu2) * s_mlp[None, :, None] + t_mlp[None, :, None]
    y2 = jax.nn.relu(y2)                                        # [B,2C,H]
    out = jnp.einsum('co,boh->bch', w_mlp2.astype(bf), y2.astype(bf),
                     preferred_element_type=jnp.float32) + in2
    return out


_COMPILED = {}
_PARAM_CACHE = {}


def _get_compiled(use_psum):
    key = bool(use_psum)
    if key not in _COMPILED:
        fn = jax.pmap(
            partial(_shard_body, use_psum=key),
            axis_name='cores',
            in_axes=(0,) * 17,
            devices=jax.devices()[:NCORES],
        )
        _COMPILED[key] = fn
    return _COMPILED[key]


def _replicated_params(params):
    """Place the (small, replicated) parameter arrays on all 8 devices once;
    reuse across calls so only x is transferred per invocation."""
    key = "params"
    cached = _PARAM_CACHE.get(key)
    if cached is not None and all(
            np.array_equal(c_host, p) for c_host, p in zip(cached[0], params)):
        return cached[1]
    devs = jax.devices()[:NCORES]
    placed = tuple(
        jax.device_put_replicated(jnp.asarray(p, jnp.float32), devs)
        for p in params)
    _PARAM_CACHE[key] = ([np.asarray(p, np.float32) for p in params], placed)
    return placed


def _prep_params(w_qkv, bn_qkv_g, bn_qkv_b, ln_g, ln_b, bn_sim_g, bn_sim_b,
                 relative, w_fc, w_mlp1, w_mlp2):
    """Host-side precomputation of folded / expanded parameters."""
    w_qkv = np.asarray(w_qkv, np.float32)
    ln_g = np.asarray(ln_g, np.float32)
    ln_b = np.asarray(ln_b, np.float32)
    w_mlp1 = np.asarray(w_mlp1, np.float32)
    relative = np.asarray(relative, np.float32)

    w_qkv_g = w_qkv * ln_g[None, :]
    s_qkv = w_qkv_g.sum(axis=1)
    t_qkv = w_qkv @ ln_b
    w_mlp1_g = w_mlp1 * ln_g[None, :]
    s_mlp = w_mlp1_g.sum(axis=1)
    t_mlp = w_mlp1 @ ln_b

    # relative position embedding: all_emb[c,i,j] = relative[c, i-j+H-1]
    ar = np.arange(H)
    ridx = ar[:, None] - ar[None, :] + H - 1
    all_emb = relative[:, ridx]                  # [2gp, H, H]
    q_emb = all_emb[: GP // 2]
    k_emb = np.ascontiguousarray(all_emb[GP // 2: GP].transpose(0, 2, 1))
    v_emb = all_emb[GP:]

    w_fc = np.asarray(w_fc, np.float32).reshape(C, GROUPS, GP, 2)
    w_fc_sv = np.ascontiguousarray(w_fc[:, :, :, 0])
    w_fc_sve = np.ascontiguousarray(w_fc[:, :, :, 1])
    return (w_qkv_g, s_qkv, t_qkv, bn_qkv_g, bn_qkv_b, bn_sim_g, bn_sim_b,
            q_emb, k_emb, v_emb, w_fc_sv, w_fc_sve, w_mlp1_g, s_mlp, t_mlp,
            w_mlp2)


def kernel(x, w_qkv, bn_qkv_g, bn_qkv_b, ln_g, ln_b, bn_sim_g, bn_sim_b,
           relative, w_fc, w_mlp1, w_mlp2):
    x = np.asarray(x, dtype=np.float32)
    # [N,C,D,H,W] -> [N,D,W,C,H] -> [B, C, H], shard B over 8 cores
    xb = np.ascontiguousarray(
        np.transpose(x, (0, 2, 4, 1, 3))).reshape(B, C, H)
    xb_sh = xb.reshape(NCORES, BL, C, H)

    params = _replicated_params(_prep_params(
        w_qkv, bn_qkv_g, bn_qkv_b, ln_g, ln_b, bn_sim_g, bn_sim_b,
        relative, w_fc, w_mlp1, w_mlp2))
    args = (jnp.asarray(xb_sh),) + params

    try:
        out_sh = _get_compiled(True)(*args)
        out_sh = np.asarray(jax.device_get(out_sh))
    except Exception:
        # collectives unavailable -> per-shard BN stats (see sharding_hint)
        out_sh = np.asarray(jax.device_get(_get_compiled(False)(*args)))

    so = out_sh.reshape(B, C, H)
    out = so.reshape(N, D, W, C, H)
    return np.ascontiguousarray(np.transpose(out, (0, 3, 1, 4, 2)))


if __name__ == "__main__":
    import reference as R
    inp = R.setup_inputs()
    inp = {k: np.asarray(v) for k, v in inp.items()}
    out = kernel(**inp)
    print("kernel output:", out.shape, out.dtype)


# revision 9
# speedup vs baseline: 12.0911x; 2.0326x over previous
"""Trainium2 kernel for nn_AxialAttention_45749991637536.

Data-parallel across the flattened axial batch B = N*D*W = 896 -> 112 rows
per NeuronCore (spec sharding_hint), params replicated, exact BatchNorm
batch statistics via cross-device all-reduce (jax.lax.pmean).

v2: restructured for the Neuron backend to remove device transposes:
  - LayerNorm folded into the qkv / mlp1 matmuls (w' = w * ln_g on host;
    the per-position mean/rstd enter as rank-1 corrections), so no
    [B,C,H] <-> [B,H,C] swapaxes are materialized on device.
  - The stacked-BN + concat over [qk,qr,kr] is replaced by per-part BN
    statistics and one fused affine-sum (BN is per-channel affine, and
    transpose commutes with it), avoiding the [B,24,56,56] concat.
  - relative-embedding tables are expanded on host (input re-indexing).
"""

import numpy as np
import jax
import jax.numpy as jnp
from functools import partial

GROUPS = 8
EPS_LN = 1e-6
EPS_BN = 1e-5

# Hardcoded problem shapes (self-contained; do not read spec.json).
N, C, D, H, W = 2, 128, 8, 56, 56
NCORES = 8
B = N * D * W            # 896
BL = B // NCORES         # 112 per core
GP = C // GROUPS         # 16


def _shard_body(xb, w_qkv_g, s_qkv, t_qkv, bn_qkv_g, bn_qkv_b,
                bn_sim_g, bn_sim_b, q_emb, k_emb, v_emb, w_fc, w_fc_sve,
                w_mlp1_g, s_mlp, t_mlp, w_mlp2, use_psum):
    """Process one shard xb: [BL, C, H]. Runs under pmap axis 'cores'."""
    Bs = xb.shape[0]
    G = GROUPS
    gp = GP
    in_x = xb

    # ---- pre-norm over channels, folded into the qkv matmul ----
    # LN(x)[c] = (x-mu)*r*g[c] + b[c]; qkv = W @ LN(x)
    #          = r * (Wg @ x) - (r*mu) * s + t     (Wg, s, t host-precomputed)
    mu = jnp.mean(xb, axis=1, keepdims=True)                    # [B,1,H]
    var = jnp.mean(jnp.square(xb), axis=1, keepdims=True) - jnp.square(mu)
    r = jax.lax.rsqrt(var + EPS_LN)                             # [B,1,H]
    bf = jnp.bfloat16
    y = jnp.einsum('oc,bch->boh', w_qkv_g.astype(bf), xb.astype(bf))
    qkv = (r.astype(bf) * y
           - (r * mu).astype(bf) * s_qkv.astype(bf)[None, :, None]
           + t_qkv.astype(bf)[None, :, None])                   # [B,2C,H] bf16

    # ---- BatchNorm1d over (B,H) with (optionally global) batch stats ----
    bmu = jnp.mean(qkv, axis=(0, 2), dtype=jnp.float32)
    bm2 = jnp.mean(jnp.square(qkv.astype(jnp.float32)), axis=(0, 2))
    if use_psum:
        bmu = jax.lax.pmean(bmu, axis_name='cores')
        bm2 = jax.lax.pmean(bm2, axis_name='cores')
    bvar = bm2 - jnp.square(bmu)
    brs = jax.lax.rsqrt(bvar + EPS_BN) * bn_qkv_g
    qkv = (qkv * brs.astype(bf)[None, :, None]
           + (bn_qkv_b - bmu * brs).astype(bf)[None, :, None])

    qkv = qkv.reshape(Bs, G, 2 * gp, H)
    q = qkv[:, :, : gp // 2].astype(bf)   # [B,G,8,H]
    k = qkv[:, :, gp // 2: gp].astype(bf)
    v = qkv[:, :, gp:].astype(bf)         # [B,G,16,H]

    # ---- similarity logits (bf16 storage, fp32 accumulation) ----
    qr = jnp.einsum('bgci,cij->bgij', q, q_emb.astype(bf))
    # k_emb is host-transposed: k_emb[c,i,j] = all_emb_k[c,j,i], so this
    # equals swapaxes(einsum('bgci,cij->bgij', k, all_emb_k), 2, 3).
    kr = jnp.einsum('bgcj,cij->bgij', k, k_emb.astype(bf))
    qk = jnp.einsum('bgci,bgcj->bgij', q, k)

    # ---- BN over the stacked [qk,qr,kr] (channel axis 3G), fused ----
    # BN is per-channel affine; stats per part, then one affine-sum.
    def _stats(tt):
        m = jnp.mean(tt, axis=(0, 2, 3), dtype=jnp.float32)
        m2 = jnp.mean(jnp.square(tt.astype(jnp.float32)), axis=(0, 2, 3))
        return m, m2
    m_qk, m2_qk = _stats(qk)
    m_qr, m2_qr = _stats(qr)
    m_kr, m2_kr = _stats(kr)
    if use_psum:
        st = jnp.stack([m_qk, m_qr, m_kr, m2_qk, m2_qr, m2_kr])
        st = jax.lax.pmean(st, axis_name='cores')
        m_qk, m_qr, m_kr, m2_qk, m2_qr, m2_kr = st
    g_s = bn_sim_g.reshape(3, G)
    b_s = bn_sim_b.reshape(3, G)
    a1 = jax.lax.rsqrt(m2_qk - jnp.square(m_qk) + EPS_BN) * g_s[0]
    a2 = jax.lax.rsqrt(m2_qr - jnp.square(m_qr) + EPS_BN) * g_s[1]
    a3 = jax.lax.rsqrt(m2_kr - jnp.square(m_kr) + EPS_BN) * g_s[2]
    cc = (b_s[0] - m_qk * a1) + (b_s[1] - m_qr * a2) + (b_s[2] - m_kr * a3)
    logits = (qk * a1.astype(bf)[None, :, None, None]
              + qr * a2.astype(bf)[None, :, None, None]
              + kr * a3.astype(bf)[None, :, None, None]
              + cc.astype(bf)[None, :, None, None])

    # softmax without max-subtraction: logits are BN-standardized per
    # channel (unit variance), so exp cannot overflow in fp32.
    e = jnp.exp(logits.astype(jnp.float32))
    rs = jnp.reciprocal(jnp.sum(e, axis=3)).astype(bf)   # [B,G,H] (over j)

    # ---- attend (softmax division folded into the small outputs) ----
    e_h = e.astype(bf)
    sv = jnp.einsum('bgij,bgcj->bgci', e_h, v) * rs[:, :, None, :]
    sve = (jnp.einsum('bgij,cij->bgci', e_h, v_emb.astype(bf))
           * rs[:, :, None, :])

    # fc branch: Linear(2C -> C) on concat([sv,sve],-1).reshape(B,2C,H),
    # with the concat folded away: channel 32g+2c+half of the view holds
    # sv/sve[g,c], so w_fc is host-split (w_fc_sv/w_fc_sve: [C,G,gp]).
    so = (jnp.einsum('bgci,ogc->bio', sv, w_fc.astype(bf),
                     preferred_element_type=jnp.float32)
          + jnp.einsum('bgci,ogc->bio', sve, w_fc_sve.astype(bf),
                       preferred_element_type=jnp.float32))
    so = so.reshape(Bs, C, H)  # memory reinterpret, matches torch .view
    so = in_x + so

    # ---- mlp branch with shared LayerNorm, LN folded into mlp1 ----
    in2 = so
    mu2 = jnp.mean(so, axis=1, keepdims=True)
    var2 = jnp.mean(jnp.square(so), axis=1, keepdims=True) - jnp.square(mu2)
    r2 = jax.lax.rsqrt(var2 + EPS_LN)
    y2 = jnp.einsum('oc,bch->boh', w_mlp1_g.astype(bf), so.astype(bf),
                    preferred_element_type=jnp.float32)
    y2 = r2 * y2 - (r2 * mu2) * s_mlp[None, :, None] + t_mlp[None, :, None]
    y2 = jax.nn.relu(y2)                                        # [B,2C,H]
    out = jnp.einsum('co,boh->bch', w_mlp2.astype(bf), y2.astype(bf),
                     preferred_element_type=jnp.float32) + in2
    return out


_COMPILED = {}
_PARAM_CACHE = {}


def _get_compiled(use_psum):
    key = bool(use_psum)
    if key not in _COMPILED:
        fn = jax.pmap(
            partial(_shard_body, use_psum=key),
            axis_name='cores',
            in_axes=(0,) * 17,
            devices=jax.devices()[:NCORES],
        )
        _COMPILED[key] = fn
    return _COMPILED[key]


def _replicated_params(params):
    """Place the (small, replicated) parameter arrays on all 8 devices once;
    reuse across calls so only x is transferred per invocation."""
    key = "params"
    cached = _PARAM_CACHE.get(key)
    if cached is not None and all(
            np.array_equal(c_host, p) for c_host, p in zip(cached[0], params)):
        return cached[1]
    devs = jax.devices()[:NCORES]
    placed = tuple(
        jax.device_put_replicated(jnp.asarray(p, jnp.float32), devs)
        for p in params)
    _PARAM_CACHE[key] = ([np.asarray(p, np.float32) for p in params], placed)
    return placed


def _prep_params(w_qkv, bn_qkv_g, bn_qkv_b, ln_g, ln_b, bn_sim_g, bn_sim_b,
                 relative, w_fc, w_mlp1, w_mlp2):
    """Host-side precomputation of folded / expanded parameters."""
    w_qkv = np.asarray(w_qkv, np.float32)
    ln_g = np.asarray(ln_g, np.float32)
    ln_b = np.asarray(ln_b, np.float32)
    w_mlp1 = np.asarray(w_mlp1, np.float32)
    relative = np.asarray(relative, np.float32)

    w_qkv_g = w_qkv * ln_g[None, :]
    s_qkv = w_qkv_g.sum(axis=1)
    t_qkv = w_qkv @ ln_b
    w_mlp1_g = w_mlp1 * ln_g[None, :]
    s_mlp = w_mlp1_g.sum(axis=1)
    t_mlp = w_mlp1 @ ln_b

    # relative position embedding: all_emb[c,i,j] = relative[c, i-j+H-1]
    ar = np.arange(H)
    ridx = ar[:, None] - ar[None, :] + H - 1
    all_emb = relative[:, ridx]                  # [2gp, H, H]
    q_emb = all_emb[: GP // 2]
    k_emb = np.ascontiguousarray(all_emb[GP // 2: GP].transpose(0, 2, 1))
    v_emb = all_emb[GP:]

    w_fc = np.asarray(w_fc, np.float32).reshape(C, GROUPS, GP, 2)
    w_fc_sv = np.ascontiguousarray(w_fc[:, :, :, 0])
    w_fc_sve = np.ascontiguousarray(w_fc[:, :, :, 1])
    return (w_qkv_g, s_qkv, t_qkv, bn_qkv_g, bn_qkv_b, bn_sim_g, bn_sim_b,
            q_emb, k_emb, v_emb, w_fc_sv, w_fc_sve, w_mlp1_g, s_mlp, t_mlp,
            w_mlp2)


def kernel(x, w_qkv, bn_qkv_g, bn_qkv_b, ln_g, ln_b, bn_sim_g, bn_sim_b,
           relative, w_fc, w_mlp1, w_mlp2):
    x = np.asarray(x, dtype=np.float32)
    # [N,C,D,H,W] -> [N,D,W,C,H] -> [B, C, H], shard B over 8 cores
    xb = np.ascontiguousarray(
        np.transpose(x, (0, 2, 4, 1, 3))).reshape(B, C, H)
    xb_sh = xb.reshape(NCORES, BL, C, H)

    params = _replicated_params(_prep_params(
        w_qkv, bn_qkv_g, bn_qkv_b, ln_g, ln_b, bn_sim_g, bn_sim_b,
        relative, w_fc, w_mlp1, w_mlp2))
    args = (jnp.asarray(xb_sh),) + params

    try:
        out_sh = _get_compiled(True)(*args)
        out_sh = np.asarray(jax.device_get(out_sh))
    except Exception:
        # collectives unavailable -> per-shard BN stats (see sharding_hint)
        out_sh = np.asarray(jax.device_get(_get_compiled(False)(*args)))

    so = out_sh.reshape(B, C, H)
    out = so.reshape(N, D, W, C, H)
    return np.ascontiguousarray(np.transpose(out, (0, 3, 1, 4, 2)))


if __name__ == "__main__":
    import reference as R
    inp = R.setup_inputs()
    inp = {k: np.asarray(v) for k, v in inp.items()}
    out = kernel(**inp)
    print("kernel output:", out.shape, out.dtype)
